# revision 25
# baseline (speedup 1.0000x reference)
"""Distributed single-head attention block for one TRN2 chip (8 NeuronCores).

Math (per batch b):  Q = x@Wq.T, K = x@Wk.T, V = x@Wv.T,
                     out = softmax(Q K^T / sqrt(D)) V
Shapes: x [4, 4096, 256], W* [256, 256], out [4, 4096, 256] (f32).

Sharding: core c handles batch b = c//2, query half qc = c%2 (2048 queries),
with full K/V for that batch (K/V projection recomputed on both cores of a
batch pair -- it is tiny). All matmul inputs are pre-transposed & bf16-cast on
the host so that no on-chip transposes are needed.

Attention is permutation-invariant over keys, so each core receives x^T
ROTATED so that its own query half occupies columns [0:2048] -- Q projects
straight from the head of the same buffer K/V project from.

Schedule (v2):
  - input DMA split across 3 engine rings (sync/scalar/gpsimd) with the two
    head pieces sized to the minimum the first Q^T accumulation needs, and
    the x tail in 256KB chunks issued in consumption order.
  - attention for the first q-tile is interleaved into the projection loop
    (one 4-keyblock group per x chunk) so PE never waits on DMA or eviction
    engines: scores can run whenever projections would stall.
  - PSUM evictions are split across DVE and ACT so neither engine's
    eviction backlog stalls the ps-pool rotation.
  - scores are computed *transposed* (tiles [k=128, q=512]): PE matmul with
    lhsT = K^T tile, rhs = Q^T tile; exp runs on ScalarE straight out of
    PSUM (scale=1/16 folded in, no max subtraction: |scores| <= ~11).
  - attn^T tiles feed the AV matmul as the stationary operand with V [k, d]
    as moving; a ones column appended to V makes the same PSUM accumulation
    produce the softmax denominator.
  - per q-tile the four normalized sub-tiles are written with a single
    coalesced DMA; the final q-tile drains its AV pipeline eagerly and
    scatters its four sub-tiles over four engine rings to shorten the tail.
"""

import os
import sys
from contextlib import ExitStack

sys.path.insert(0, "/opt/trn_rl_repo")

import numpy as np
import ml_dtypes

B, S, D = 4, 4096, 256
NCORES = 8
SQ = S // 2  # queries per core
P = 128  # SBUF partitions
EB = D // P  # e (contraction) blocks for projections
DB = D // P  # d blocks
KB = S // P  # key blocks of 128
QT = 512  # q tile (matmul moving free dim)
NQB = SQ // QT  # q tiles per core
SUBQ = QT // P  # 128-query sub-blocks per q tile
HC = 512  # head chunk: x columns packed with wq
NXCH = (S - HC) // QT  # 7 tail chunks of 512 columns

LAST_RESULT = None  # BassKernelResults of the most recent run (for test.py)
_CACHE = {}


def _build_nc():
    import concourse.tile as tile
    from concourse import bacc, mybir

    bf16 = mybir.dt.bfloat16
    f32 = mybir.dt.float32
    Exp = mybir.ActivationFunctionType.Exp
    Copy = mybir.ActivationFunctionType.Copy

    nc = bacc.Bacc(None, target_bir_lowering=False)

    # head0: [wq_e0(256) | x_e0[:,0:512](512) | wq_e1(256) | x_e1[:,0:512](512)]
    head0 = nc.declare_dram_parameter("head0", [P, 2 * (2 * P + HC)], bf16,
                                      isOutput=False)
    # wkv: [wk pk (512) | wv pk (512)]
    wkv = nc.declare_dram_parameter("wkv", [P, 2 * EB * D], bf16, isOutput=False)
    # x_rest: NXCH chunks of [x_e0[:,c0:c0+512] | x_e1[:,c0:c0+512]] (1024 each)
    x_rest = nc.declare_dram_parameter("x_rest", [P, NXCH * EB * QT], bf16,
                                       isOutput=False)
    out = nc.declare_dram_parameter("out", [SQ, D], f32, isOutput=True)

    with tile.TileContext(nc) as tc, ExitStack() as ctx:
        consts = ctx.enter_context(tc.tile_pool(name="consts", bufs=1))
        ps = ctx.enter_context(tc.tile_pool(name="ps", bufs=4, space="PSUM"))
        po = ctx.enter_context(tc.tile_pool(name="po", bufs=4, space="PSUM"))
        work = ctx.enter_context(tc.tile_pool(name="work", bufs=5))
        outp = ctx.enter_context(tc.tile_pool(name="outp", bufs=2))
        normp = ctx.enter_context(tc.tile_pool(name="normp", bufs=4))

        # ---- SBUF destination tiles -----------------------------------------
        head_sb = consts.tile([P, 2 * (2 * P + HC)], bf16)
        wkv_sb = consts.tile([P, 2 * EB * D], bf16)
        x_sb = consts.tile([P, NXCH, EB, QT], bf16)  # x cols [512:4096)
        kt_sb = consts.tile([P, DB, S], bf16)  # K^T [d, k]
        qt_sb = consts.tile([P, DB, SQ], bf16)  # Q^T [d, q]
        v_sb = consts.tile([P, KB, D + 1], bf16)  # V [k, d] + ones column
        warm_l = consts.tile([P, P], bf16)
        warm_r = consts.tile([P, QT], bf16)

        # memsets first on DVE (instant; before any DMA issue occupies DVE)
        nc.vector.memset(warm_l, 0.0)
        nc.vector.memset(warm_r, 0.0)
        nc.vector.memset(v_sb[:, :, D : D + 1], 1.0)

        # ---- input DMA: 4 parallel engine rings, consumption order ----------
        # head pieces gate the first projections; x chunks follow in the order
        # the interleaved projection loop consumes them.
        HW = 2 * P + HC  # 768 cols per head piece
        nc.sync.dma_start(out=head_sb[:, :HW], in_=head0[:, :HW])
        nc.scalar.dma_start(out=head_sb[:, HW:], in_=head0[:, HW:])
        nc.gpsimd.dma_start(out=wkv_sb, in_=wkv[:, :])

        def xch(i):  # chunk i covers x cols [512*(i+1), 512*(i+2))
            return (
                x_sb[:, i],
                x_rest[:, i * EB * QT : (i + 1) * EB * QT].rearrange(
                    "p (a m) -> p a m", a=EB
                ),
            )

        for eng, i in [
            (nc.sync, 0),
            (nc.scalar, 1),
            (nc.gpsimd, 2),
            (nc.sync, 3),
            (nc.scalar, 4),
            (nc.gpsimd, 5),
            (nc.gpsimd, 6),
        ]:
            o, s = xch(i)
            eng.dma_start(out=o, in_=s)

        def wq(ea):
            return head_sb[:, ea * HW : ea * HW + 2 * P]

        def xs(ea, c0, w):
            """x^T slice [128, w] for e-block ea, columns [c0, c0+w)."""
            if c0 + w <= HC:
                return head_sb[:, ea * HW + 2 * P + c0 : ea * HW + 2 * P + c0 + w]
            ch = c0 // QT - 1
            return x_sb[:, ch, ea, c0 % QT : c0 % QT + w]

        wk_sb = wkv_sb[:, 0 : EB * D].rearrange("p (a d) -> p a d", a=EB)
        wv_sb = wkv_sb[:, EB * D :].rearrange("p (a d) -> p a d", a=EB)

        # ---- PE warmup: dummy matmuls while the first DMAs land, so HAM
        # un-throttles (1.2 -> 2.4 GHz) by the time the projections run.
        for _ in range(5):
            wp = ps.tile([P, QT], f32, name="wp", tag="pt")
            nc.tensor.matmul(wp, lhsT=warm_l, rhs=warm_r, start=True, stop=True)

        # ---- projection pieces ----------------------------------------------
        # PSUM evictions spread over DVE / ACT / GPSIMD: any single engine
        # alone is slower than PE through this phase.
        def qt_part(kc, da):
            sl = slice(kc * QT, (kc + 1) * QT)
            pt = ps.tile([P, QT], f32, name="pt", tag="pt")
            for ea in range(EB):
                nc.tensor.matmul(
                    pt,
                    lhsT=wq(ea)[:, da * P : (da + 1) * P],
                    rhs=xs(ea, kc * QT, QT),
                    start=(ea == 0),
                    stop=(ea == EB - 1),
                )
            if da == 1:
                nc.scalar.copy(out=qt_sb[:, da, sl], in_=pt)
            else:
                nc.vector.tensor_copy(out=qt_sb[:, da, sl], in_=pt)

        def kt_part(kc, da):
            sl = slice(kc * QT, (kc + 1) * QT)
            pt = ps.tile([P, QT], f32, name="pt", tag="pt")
            for ea in range(EB):
                nc.tensor.matmul(
                    pt,
                    lhsT=wk_sb[:, ea, da * P : (da + 1) * P],
                    rhs=xs(ea, kc * QT, QT),
                    start=(ea == 0),
                    stop=(ea == EB - 1),
                )
            if da == 1:
                nc.scalar.copy(out=kt_sb[:, da, sl], in_=pt)
            else:
                nc.vector.tensor_copy(out=kt_sb[:, da, sl], in_=pt)

        def v_part(kb):
            pt = ps.tile([P, QT], f32, name="pt", tag="pt")
            for ea in range(EB):
                nc.tensor.matmul(
                    pt[:, :D],
                    lhsT=xs(ea, kb * P, P),
                    rhs=wv_sb[:, ea, :],
                    start=(ea == 0),
                    stop=(ea == EB - 1),
                )
            if kb % 2 == 1:
                nc.scalar.copy(out=v_sb[:, kb, 0:D], in_=pt[:, :D])
            else:
                nc.vector.tensor_copy(out=v_sb[:, kb, 0:D], in_=pt[:, :D])

        def proj(kc):
            # sandwich every V matmul between 512-wide Q^T/K^T streams so
            # each V LDWEIGHTS prefetches fully under a long stream
            kb0 = kc * (QT // P)
            if kc * QT < SQ:
                qt_part(kc, 0)
                v_part(kb0)
                kt_part(kc, 0)
                v_part(kb0 + 1)
                qt_part(kc, 1)
                v_part(kb0 + 2)
                kt_part(kc, 1)
                v_part(kb0 + 3)
            else:
                kt_part(kc, 0)
                v_part(kb0)
                v_part(kb0 + 1)
                kt_part(kc, 1)
                v_part(kb0 + 2)
                v_part(kb0 + 3)

        # ---- attention pieces -----------------------------------------------
        inv_sqrt_d = 1.0 / np.sqrt(D)
        state = {}

        def qb_begin(qb):
            state["po"] = [
                po.tile([P, D + 1], f32, name="po_acc", tag="po_acc")
                for _ in range(SUBQ)
            ]
            state["pend"] = []

        def emit_av(at, kb):
            for sub in range(SUBQ):
                nc.tensor.matmul(
                    state["po"][sub],
                    lhsT=at[:, sub * P : (sub + 1) * P],
                    rhs=v_sb[:, kb, :],
                    start=(kb == 0),
                    stop=(kb == KB - 1),
                )

        def attn_kb(qb, kb, depth=4):
            pt = ps.tile([P, QT], f32)
            for da in range(DB):
                nc.tensor.matmul(
                    pt,
                    lhsT=kt_sb[:, da, kb * P : (kb + 1) * P],
                    rhs=qt_sb[:, da, qb * QT : (qb + 1) * QT],
                    start=(da == 0),
                    stop=(da == DB - 1),
                )
            at = work.tile([P, QT], bf16)
            nc.scalar.activation(out=at, in_=pt, func=Exp, scale=inv_sqrt_d)
            pend = state["pend"]
            pend.append((at, kb))
            while len(pend) > depth:
                emit_av(*pend.pop(0))

        def qb_end(qb):
            for at, kb in state["pend"]:
                emit_av(at, kb)
            state["pend"] = []
            po_tiles = state["po"]
            ob = outp.tile([P, SUBQ, D], f32)
            last = qb == NQB - 1
            for sub in range(SUBQ):
                rc = normp.tile([P, 1], f32)
                nc.vector.reciprocal(out=rc, in_=po_tiles[sub][:, D : D + 1])
                # on the final q-block, split the normalize multiplies across
                # DVE and ACT to halve the kernel tail; mid-kernel keep them
                # on DVE (ACT-side normalize delays PSUM release for next qb)
                if last and sub % 2 == 1:
                    nc.scalar.activation(
                        out=ob[:, sub, :], in_=po_tiles[sub][:, 0:D],
                        func=Copy, scale=rc,
                    )
                else:
                    nc.vector.tensor_scalar_mul(
                        ob[:, sub, :], po_tiles[sub][:, 0:D], rc
                    )
            r0 = qb * QT
            if last:
                # parallel engine rings to drain the tail fast
                engs = [nc.sync, nc.gpsimd, nc.scalar, nc.sync]
                for sub in range(SUBQ):
                    engs[sub].dma_start(
                        out=out[r0 + sub * P : r0 + (sub + 1) * P, :],
                        in_=ob[:, sub, :],
                    )
            else:
                eng = [nc.sync, nc.gpsimd, nc.sync][qb]
                eng.dma_start(
                    out=out[r0 : r0 + QT, :].rearrange("(s p) d -> p s d", p=P),
                    in_=ob,
                )

        # ---- schedule -------------------------------------------------------
        proj(0)
        proj(1)
        qb_begin(0)
        for kc in range(2, S // QT):  # interleave: attn group then proj chunk
            for kb in range(4 * (kc - 2), 4 * (kc - 1)):
                attn_kb(0, kb)
            proj(kc)
        for kb in range(4 * (S // QT - 2), KB):
            attn_kb(0, kb)
        qb_end(0)

        for qb in range(1, NQB):
            qb_begin(qb)
            for kb in range(KB):
                depth = 2 if (qb == NQB - 1 and kb >= KB - 4) else 4
                attn_kb(qb, kb, depth=depth)
            qb_end(qb)

    nc.finalize()
    return nc


def _ensure_ntff_hook():
    """This image's antenv lacks axon_hooks; synthesize it from the ctypes
    implementation in trn_agent_boot so trace=True can capture NTFF profiles."""
    import types

    try:
        from antenv.axon_hooks import get_axon_ntff_profile_hook  # noqa: F401

        return
    except ImportError:
        pass
    import antenv  # noqa: F401
    from trn_agent_boot.trn_boot import _ntff_profile_via_ctypes

    hook = _ntff_profile_via_ctypes("/opt/axon/libaxon_pjrt.so")
    mod = types.ModuleType("antenv.axon_hooks")
    mod.get_axon_ntff_profile_hook = lambda: hook
    mod.set_axon_ntff_profile_hook = lambda h: None
    sys.modules["antenv.axon_hooks"] = mod


def kernel(x, Wq, Wk, Wv):
    from concourse.bass_utils import run_bass_kernel_spmd

    global LAST_RESULT
    if "nc" not in _CACHE:
        _CACHE["nc"] = _build_nc()
    nc = _CACHE["nc"]

    bf = ml_dtypes.bfloat16
    x = np.asarray(x, dtype=np.float32)
    xT = np.ascontiguousarray(x.transpose(0, 2, 1)).astype(bf)  # [B, D, S]
    wqt = np.asarray(Wq, np.float32).T.astype(bf)
    wkt = np.asarray(Wk, np.float32).T.astype(bf)
    wvt = np.asarray(Wv, np.float32).T.astype(bf)

    def pk(a2d):  # [256, w] -> [128, 2*w] (e-blocks adjacent per partition)
        w = a2d.shape[1]
        return a2d.reshape(2, P, w).transpose(1, 0, 2).reshape(P, 2 * w)

    wq_e = wqt.reshape(2, P, D)  # [ea][p][d]
    wkv_np = np.ascontiguousarray(np.concatenate([pk(wkt), pk(wvt)], axis=1))

    in_maps = []
    for c in range(NCORES):
        b, qc = c // 2, c % 2
        if qc == 0:
            xr = xT[b]
        else:
            # rotate so this core's query half occupies columns [0:SQ);
            # key order is irrelevant to softmax attention.
            xr = np.concatenate([xT[b][:, SQ:], xT[b][:, :SQ]], axis=1)
        xe = xr.reshape(2, P, S)  # [ea][p][col]
        head = np.ascontiguousarray(
            np.concatenate(
                [wq_e[0], xe[0][:, 0:HC], wq_e[1], xe[1][:, 0:HC]], axis=1
            )
        )
        rest = np.ascontiguousarray(
            np.concatenate(
                [
                    np.concatenate(
                        [xe[0][:, c0 : c0 + QT], xe[1][:, c0 : c0 + QT]], axis=1
                    )
                    for c0 in range(HC, S, QT)
                ],
                axis=1,
            )
        )
        in_maps.append({"head0": head, "wkv": wkv_np, "x_rest": rest})

    trace = bool(int(os.environ.get("KERNEL_TRACE", "0")))
    if trace:
        _ensure_ntff_hook()
    LAST_RESULT = run_bass_kernel_spmd(
        nc, in_maps, core_ids=list(range(NCORES)), trace=trace
    )
    outs = [LAST_RESULT.results[c]["out"] for c in range(NCORES)]
    full = np.empty((B, S, D), dtype=np.float32)
    for c in range(NCORES):
        b, qc = c // 2, c % 2
        full[b, qc * SQ : (qc + 1) * SQ, :] = outs[c]
    return full


# revision 26
# speedup vs baseline: 1.2038x; 1.2038x over previous
"""Distributed single-head attention block for one TRN2 chip (8 NeuronCores).

Math (per batch b):  Q = x@Wq.T, K = x@Wk.T, V = x@Wv.T,
                     out = softmax(Q K^T / sqrt(D)) V
Shapes: x [4, 4096, 256], W* [256, 256], out [4, 4096, 256] (f32).

Sharding: core c handles batch b = c//2, query half qc = c%2 (2048 queries),
with full K/V for that batch (K/V projection recomputed on both cores of a
batch pair -- it is tiny). All matmul inputs are pre-transposed & bf16-cast on
the host so that no on-chip transposes are needed.

Attention is permutation-invariant over keys, so each core receives x^T
ROTATED so that its own query half occupies columns [0:2048] -- Q projects
straight from the head of the same buffer K/V project from.

Schedule (v2):
  - input DMA split across 3 engine rings (sync/scalar/gpsimd) with the two
    head pieces sized to the minimum the first Q^T accumulation needs, and
    the x tail in 256KB chunks issued in consumption order.
  - attention for the first q-tile is interleaved into the projection loop
    (one 4-keyblock group per x chunk) so PE never waits on DMA or eviction
    engines: scores can run whenever projections would stall.
  - PSUM evictions are split across DVE and ACT so neither engine's
    eviction backlog stalls the ps-pool rotation.
  - scores are computed *transposed* (tiles [k=128, q=512]): PE matmul with
    lhsT = K^T tile, rhs = Q^T tile; exp runs on ScalarE straight out of
    PSUM (scale=1/16 folded in, no max subtraction: |scores| <= ~11).
  - attn^T tiles feed the AV matmul as the stationary operand with V [k, d]
    as moving; a ones column appended to V makes the same PSUM accumulation
    produce the softmax denominator.
  - per q-tile the four normalized sub-tiles are written with a single
    coalesced DMA; the final q-tile drains its AV pipeline eagerly and
    scatters its four sub-tiles over four engine rings to shorten the tail.
"""

import os
import sys
from contextlib import ExitStack

sys.path.insert(0, "/opt/trn_rl_repo")

import numpy as np
import ml_dtypes

B, S, D = 4, 4096, 256
NCORES = 8
SQ = S // 2  # queries per core
P = 128  # SBUF partitions
EB = D // P  # e (contraction) blocks for projections
DB = D // P  # d blocks
KB = S // P  # key blocks of 128
QT = 512  # q tile (matmul moving free dim)
NQB = SQ // QT  # q tiles per core
SUBQ = QT // P  # 128-query sub-blocks per q tile
HC = 512  # head chunk: x columns packed with wq
NXCH = (S - HC) // QT  # 7 tail chunks of 512 columns

LAST_RESULT = None  # BassKernelResults of the most recent run (for test.py)
_CACHE = {}


def _build_nc():
    import concourse.tile as tile
    from concourse import bacc, mybir

    bf16 = mybir.dt.bfloat16
    f32 = mybir.dt.float32
    Exp = mybir.ActivationFunctionType.Exp
    Copy = mybir.ActivationFunctionType.Copy

    nc = bacc.Bacc(None, target_bir_lowering=False)

    # head0: [wq_e0(256) | x_e0[:,0:512](512) | wq_e1(256) | x_e1[:,0:512](512)]
    head0 = nc.declare_dram_parameter("head0", [P, 2 * (2 * P + HC)], bf16,
                                      isOutput=False)
    # wkv: [wk pk (512) | wv pk (512)]
    wkv = nc.declare_dram_parameter("wkv", [P, 2 * EB * D], bf16, isOutput=False)
    # x_rest: NXCH chunks of [x_e0[:,c0:c0+512] | x_e1[:,c0:c0+512]] (1024 each)
    x_rest = nc.declare_dram_parameter("x_rest", [P, NXCH * EB * QT], bf16,
                                       isOutput=False)
    out = nc.declare_dram_parameter("out", [SQ, D], f32, isOutput=True)

    with tile.TileContext(nc) as tc, ExitStack() as ctx:
        consts = ctx.enter_context(tc.tile_pool(name="consts", bufs=1))
        ps = ctx.enter_context(tc.tile_pool(name="ps", bufs=4, space="PSUM"))
        po = ctx.enter_context(tc.tile_pool(name="po", bufs=4, space="PSUM"))
        work = ctx.enter_context(tc.tile_pool(name="work", bufs=5))
        outp = ctx.enter_context(tc.tile_pool(name="outp", bufs=2))
        normp = ctx.enter_context(tc.tile_pool(name="normp", bufs=4))

        # ---- SBUF destination tiles -----------------------------------------
        head_sb = consts.tile([P, 2 * (2 * P + HC)], bf16)
        wkv_sb = consts.tile([P, 2 * EB * D], bf16)
        x_sb = consts.tile([P, NXCH, EB, QT], bf16)  # x cols [512:4096)
        kt_sb = consts.tile([P, DB, S], bf16)  # K^T [d, k]
        qt_sb = consts.tile([P, DB, SQ], bf16)  # Q^T [d, q]
        v_sb = consts.tile([P, KB, D + 1], bf16)  # V [k, d] + ones column
        warm_l = consts.tile([P, P], bf16)
        warm_r = consts.tile([P, QT], bf16)

        # memsets first on DVE (instant; before any DMA issue occupies DVE)
        nc.vector.memset(warm_l, 0.0)
        nc.vector.memset(warm_r, 0.0)
        nc.vector.memset(v_sb[:, :, D : D + 1], 1.0)

        # ---- input DMA: 4 parallel engine rings, consumption order ----------
        # head pieces gate the first projections; x chunks follow in the order
        # the interleaved projection loop consumes them.
        HW = 2 * P + HC  # 768 cols per head piece
        nc.sync.dma_start(out=head_sb[:, :HW], in_=head0[:, :HW])
        nc.scalar.dma_start(out=head_sb[:, HW:], in_=head0[:, HW:])
        nc.gpsimd.dma_start(out=wkv_sb, in_=wkv[:, :])

        def xch(i):  # chunk i covers x cols [512*(i+1), 512*(i+2))
            return (
                x_sb[:, i],
                x_rest[:, i * EB * QT : (i + 1) * EB * QT].rearrange(
                    "p (a m) -> p a m", a=EB
                ),
            )

        for eng, i in [
            (nc.sync, 0),
            (nc.scalar, 1),
            (nc.gpsimd, 2),
            (nc.sync, 3),
            (nc.scalar, 4),
            (nc.gpsimd, 5),
            (nc.gpsimd, 6),
        ]:
            o, s = xch(i)
            eng.dma_start(out=o, in_=s)

        def wq(ea):
            return head_sb[:, ea * HW : ea * HW + 2 * P]

        def xs(ea, c0, w):
            """x^T slice [128, w] for e-block ea, columns [c0, c0+w)."""
            if c0 + w <= HC:
                return head_sb[:, ea * HW + 2 * P + c0 : ea * HW + 2 * P + c0 + w]
            ch = c0 // QT - 1
            return x_sb[:, ch, ea, c0 % QT : c0 % QT + w]

        wk_sb = wkv_sb[:, 0 : EB * D].rearrange("p (a d) -> p a d", a=EB)
        wv_sb = wkv_sb[:, EB * D :].rearrange("p (a d) -> p a d", a=EB)

        # ---- PE warmup: dummy matmuls while the first DMAs land, so HAM
        # un-throttles (1.2 -> 2.4 GHz) by the time the projections run.
        for _ in range(5):
            wp = ps.tile([P, QT], f32, name="wp", tag="pt")
            nc.tensor.matmul(wp, lhsT=warm_l, rhs=warm_r, start=True, stop=True)

        # ---- projection pieces ----------------------------------------------
        # PSUM evictions spread over DVE / ACT / GPSIMD: any single engine
        # alone is slower than PE through this phase.
        def qt_part(kc, da):
            sl = slice(kc * QT, (kc + 1) * QT)
            pt = ps.tile([P, QT], f32, name="pt", tag="pt")
            for ea in range(EB):
                nc.tensor.matmul(
                    pt,
                    lhsT=wq(ea)[:, da * P : (da + 1) * P],
                    rhs=xs(ea, kc * QT, QT),
                    start=(ea == 0),
                    stop=(ea == EB - 1),
                )
            if da == 1:
                nc.scalar.copy(out=qt_sb[:, da, sl], in_=pt)
            else:
                nc.vector.tensor_copy(out=qt_sb[:, da, sl], in_=pt)

        def kt_part(kc, da):
            sl = slice(kc * QT, (kc + 1) * QT)
            pt = ps.tile([P, QT], f32, name="pt", tag="pt")
            for ea in range(EB):
                nc.tensor.matmul(
                    pt,
                    lhsT=wk_sb[:, ea, da * P : (da + 1) * P],
                    rhs=xs(ea, kc * QT, QT),
                    start=(ea == 0),
                    stop=(ea == EB - 1),
                )
            if da == 1:
                nc.scalar.copy(out=kt_sb[:, da, sl], in_=pt)
            else:
                nc.vector.tensor_copy(out=kt_sb[:, da, sl], in_=pt)

        def v_part(kb):
            pt = ps.tile([P, QT], f32, name="pt", tag="pt")
            for ea in range(EB):
                nc.tensor.matmul(
                    pt[:, :D],
                    lhsT=xs(ea, kb * P, P),
                    rhs=wv_sb[:, ea, :],
                    start=(ea == 0),
                    stop=(ea == EB - 1),
                )
            if kb % 2 == 1:
                nc.scalar.copy(out=v_sb[:, kb, 0:D], in_=pt[:, :D])
            else:
                nc.vector.tensor_copy(out=v_sb[:, kb, 0:D], in_=pt[:, :D])

        def proj(kc):
            # sandwich every V matmul between 512-wide Q^T/K^T streams so
            # each V LDWEIGHTS prefetches fully under a long stream
            kb0 = kc * (QT // P)
            if kc * QT < SQ:
                qt_part(kc, 0)
                v_part(kb0)
                kt_part(kc, 0)
                v_part(kb0 + 1)
                qt_part(kc, 1)
                v_part(kb0 + 2)
                kt_part(kc, 1)
                v_part(kb0 + 3)
            else:
                kt_part(kc, 0)
                v_part(kb0)
                v_part(kb0 + 1)
                kt_part(kc, 1)
                v_part(kb0 + 2)
                v_part(kb0 + 3)

        # ---- attention pieces -----------------------------------------------
        inv_sqrt_d = 1.0 / np.sqrt(D)
        state = {}

        def qb_begin(qb):
            state["po"] = [
                po.tile([P, D + 1], f32, name="po_acc", tag="po_acc")
                for _ in range(SUBQ)
            ]
            state["pend"] = []

        def emit_av(at, kb):
            for sub in range(SUBQ):
                nc.tensor.matmul(
                    state["po"][sub],
                    lhsT=at[:, sub * P : (sub + 1) * P],
                    rhs=v_sb[:, kb, :],
                    start=(kb == 0),
                    stop=(kb == KB - 1),
                )

        def attn_kb(qb, kb, depth=4):
            pt = ps.tile([P, QT], f32)
            for da in range(DB):
                nc.tensor.matmul(
                    pt,
                    lhsT=kt_sb[:, da, kb * P : (kb + 1) * P],
                    rhs=qt_sb[:, da, qb * QT : (qb + 1) * QT],
                    start=(da == 0),
                    stop=(da == DB - 1),
                )
            if qb == NQB - 1 and kb == KB - 1:
                # the final exp is on the kernel's critical path: drain the
                # pipeline, then split it into four 128-wide pieces so each
                # AV can start as soon as its quarter is ready.
                for a, k in state["pend"]:
                    emit_av(a, k)
                state["pend"] = []
                at = work.tile([P, QT], bf16)
                for sub in range(SUBQ):
                    qsl = slice(sub * P, (sub + 1) * P)
                    nc.scalar.activation(
                        out=at[:, qsl], in_=pt[:, qsl], func=Exp,
                        scale=inv_sqrt_d,
                    )
                    nc.tensor.matmul(
                        state["po"][sub],
                        lhsT=at[:, qsl],
                        rhs=v_sb[:, kb, :],
                        start=False,
                        stop=True,
                    )
                return
            at = work.tile([P, QT], bf16)
            nc.scalar.activation(out=at, in_=pt, func=Exp, scale=inv_sqrt_d)
            pend = state["pend"]
            pend.append((at, kb))
            while len(pend) > depth:
                emit_av(*pend.pop(0))

        def qb_end(qb):
            for at, kb in state["pend"]:
                emit_av(at, kb)
            state["pend"] = []
            po_tiles = state["po"]
            ob = outp.tile([P, SUBQ, D], f32)
            last = qb == NQB - 1
            engs = [nc.sync, nc.gpsimd, nc.scalar, nc.sync]
            r0 = qb * QT
            for sub in range(SUBQ):
                rc = normp.tile([P, 1], f32)
                nc.vector.reciprocal(out=rc, in_=po_tiles[sub][:, D : D + 1])
                # on the final q-block, split the normalize multiplies across
                # DVE and ACT to halve the kernel tail and issue each
                # sub-tile's DMA the moment its normalize lands; mid-kernel
                # keep them on DVE (ACT-side normalize delays PSUM release
                # for the next q-tile) and write one coalesced DMA.
                if last and sub % 2 == 1:
                    nc.scalar.activation(
                        out=ob[:, sub, :], in_=po_tiles[sub][:, 0:D],
                        func=Copy, scale=rc,
                    )
                else:
                    nc.vector.tensor_scalar_mul(
                        ob[:, sub, :], po_tiles[sub][:, 0:D], rc
                    )
                if last:
                    engs[sub].dma_start(
                        out=out[r0 + sub * P : r0 + (sub + 1) * P, :],
                        in_=ob[:, sub, :],
                    )
            if not last:
                eng = [nc.sync, nc.gpsimd, nc.sync][qb]
                eng.dma_start(
                    out=out[r0 : r0 + QT, :].rearrange("(s p) d -> p s d", p=P),
                    in_=ob,
                )

        # ---- schedule -------------------------------------------------------
        proj(0)
        proj(1)
        qb_begin(0)
        for kc in range(2, S // QT):  # interleave: attn group then proj chunk
            for kb in range(4 * (kc - 2), 4 * (kc - 1)):
                attn_kb(0, kb)
            proj(kc)
        for kb in range(4 * (S // QT - 2), KB):
            attn_kb(0, kb)
        qb_end(0)

        for qb in range(1, NQB):
            qb_begin(qb)
            for kb in range(KB):
                depth = 2 if (qb == NQB - 1 and kb >= KB - 4) else 4
                attn_kb(qb, kb, depth=depth)
            qb_end(qb)

    nc.finalize()
    return nc


def _ensure_ntff_hook():
    """This image's antenv lacks axon_hooks; synthesize it from the ctypes
    implementation in trn_agent_boot so trace=True can capture NTFF profiles."""
    import types

    try:
        from antenv.axon_hooks import get_axon_ntff_profile_hook  # noqa: F401

        return
    except ImportError:
        pass
    import antenv  # noqa: F401
    from trn_agent_boot.trn_boot import _ntff_profile_via_ctypes

    hook = _ntff_profile_via_ctypes("/opt/axon/libaxon_pjrt.so")
    mod = types.ModuleType("antenv.axon_hooks")
    mod.get_axon_ntff_profile_hook = lambda: hook
    mod.set_axon_ntff_profile_hook = lambda h: None
    sys.modules["antenv.axon_hooks"] = mod


def kernel(x, Wq, Wk, Wv):
    from concourse.bass_utils import run_bass_kernel_spmd

    global LAST_RESULT
    if "nc" not in _CACHE:
        _CACHE["nc"] = _build_nc()
    nc = _CACHE["nc"]

    bf = ml_dtypes.bfloat16
    x = np.asarray(x, dtype=np.float32)
    xT = np.ascontiguousarray(x.transpose(0, 2, 1)).astype(bf)  # [B, D, S]
    wqt = np.asarray(Wq, np.float32).T.astype(bf)
    wkt = np.asarray(Wk, np.float32).T.astype(bf)
    wvt = np.asarray(Wv, np.float32).T.astype(bf)

    def pk(a2d):  # [256, w] -> [128, 2*w] (e-blocks adjacent per partition)
        w = a2d.shape[1]
        return a2d.reshape(2, P, w).transpose(1, 0, 2).reshape(P, 2 * w)

    wq_e = wqt.reshape(2, P, D)  # [ea][p][d]
    wkv_np = np.ascontiguousarray(np.concatenate([pk(wkt), pk(wvt)], axis=1))

    in_maps = []
    for c in range(NCORES):
        b, qc = c // 2, c % 2
        if qc == 0:
            xr = xT[b]
        else:
            # rotate so this core's query half occupies columns [0:SQ);
            # key order is irrelevant to softmax attention.
            xr = np.concatenate([xT[b][:, SQ:], xT[b][:, :SQ]], axis=1)
        xe = xr.reshape(2, P, S)  # [ea][p][col]
        head = np.ascontiguousarray(
            np.concatenate(
                [wq_e[0], xe[0][:, 0:HC], wq_e[1], xe[1][:, 0:HC]], axis=1
            )
        )
        rest = np.ascontiguousarray(
            np.concatenate(
                [
                    np.concatenate(
                        [xe[0][:, c0 : c0 + QT], xe[1][:, c0 : c0 + QT]], axis=1
                    )
                    for c0 in range(HC, S, QT)
                ],
                axis=1,
            )
        )
        in_maps.append({"head0": head, "wkv": wkv_np, "x_rest": rest})

    trace = bool(int(os.environ.get("KERNEL_TRACE", "0")))
    if trace:
        _ensure_ntff_hook()
    LAST_RESULT = run_bass_kernel_spmd(
        nc, in_maps, core_ids=list(range(NCORES)), trace=trace
    )
    outs = [LAST_RESULT.results[c]["out"] for c in range(NCORES)]
    full = np.empty((B, S, D), dtype=np.float32)
    for c in range(NCORES):
        b, qc = c // 2, c % 2
        full[b, qc * SQ : (qc + 1) * SQ, :] = outs[c]
    return full
